# revision 15
# baseline (speedup 1.0000x reference)
"""TopK sparse autoencoder forward pass on 8 TRN2 NeuronCores.

Data-parallel over the batch: each core owns 512 rows and runs an identical
program (SPMD).  Per core:

  A. encode:  acts = relu((x - b_dec) @ W_enc.T + b_enc)
     - MODE bf16x3: 3-term bf16 hi/lo split (xh@Wh + xh@Wl + xl@Wh), ~fp32
       precision -> exact top-k selection.
     - MODE fp16x2: 2-term fp16 x-split ((xh + xl) @ W16); residual x@(W -
       W16) gives acts error ~1e-4 -> a few tens of swapped rows batch-wide,
       rel err ~1.5e-2 (gate 2e-2); W streamed once at half the HBM traffic
       and 2/3 the PE time of bf16x3.
     - W streamed from HBM once; fp32 acts spilled to DRAM
     - per-256-chunk top-8 candidates extracted from drain bounces (DVE max8)
  B. topk: exact threshold tau from the candidate array via iterated
     max8 + match_replace, split hierarchically: candidates of chunks 0..79
     are reduced to a top-8R block on the DVE *while the PE still encodes*
     chunks 80..95; the post-encode stage only reduces 128+8R columns.
  C. mask: enc = (acts >= tau) * acts, cast bf16, DMA-transposed to [F, B]
  D. decode: x_hat = enc @ W_dec.T + b_dec  [bf16, encoded-stationary],
     block-pipelined over 2048-wide F blocks with one-block lookahead on
     the acts reload + mask + transpose chain (1024-wide granules) and
     W_dec prefetch, so the PE never idles between blocks (idle gaps ->
     HAM re-throttle to half clock, which is what made the un-pipelined
     version slow).

The per-chunk top-8 candidate set provably contains the true top-k unless
some 256-wide chunk holds >8 of the top-k values; that condition is detected
on-device (flag = chunk-8th-largest > tau) and the handful of flagged rows
(expected: zero) are recomputed exactly on the host.
"""

import numpy as np
import ml_dtypes

ACT_DIM = 768
DICT = 24576
BATCH = 4096
NCORES = 8
ROWS = BATCH // NCORES          # 512 rows per core
NT = ROWS // 128                # 4 row-tiles per core
CH = 256                        # stage-1 chunk width
NCH = DICT // CH                # 96 chunks
CANDW = NCH * 8                 # 768 candidates per row
NEG = -1.0e30
BF16 = ml_dtypes.bfloat16
NA = ACT_DIM // 128             # 6 K-chunks
NSC = DICT // 512               # 48 encode column-chunks
# layer-2 tau tournament: 96 chunks -> 32 groups of 3 chunks (768 dict
# cols); per-group top-8 extracted incrementally during encode, stage 2
# reduces the 256-wide layer-2 array after encode.  A group holding >8
# of the top-k is detected via the flag (expected ~30 rows batch-wide,
# host-repaired exactly).
GRP = 3                         # chunks per layer-2 group
NGRP = NCH // GRP               # 32 groups
L2W = NGRP * 8                  # 256 layer-2 candidate cols
NBLK = DICT // 2048             # 12 C/D blocks
NF = DICT // 128                # 192 decoder f-chunks

MODE = "fp16x2"                 # "bf16x3" | "fp16x2"

_cache = {}


def _build(k: int, with_benc: bool, mode: str):
    import concourse.bass as bass
    import concourse.mybir as mybir
    from concourse import bacc
    from concourse import tile

    f32 = mybir.dt.float32
    bf16 = mybir.dt.bfloat16
    fp16 = mybir.dt.float16
    ROUNDS = (k + 7) // 8
    R8 = 8 * ROUNDS

    nc = bacc.Bacc("TRN2", target_bir_lowering=False, debug=False,
                   num_devices=NCORES)

    wdt = bf16 if mode == "bf16x3" else fp16
    xh_d = nc.dram_tensor("xh", [ACT_DIM, ROWS], wdt, kind="ExternalInput")
    xl_d = nc.dram_tensor("xl", [ACT_DIM, ROWS], wdt, kind="ExternalInput")
    wh_d = nc.dram_tensor("wencH", [ACT_DIM, DICT], wdt, kind="ExternalInput")
    if mode == "bf16x3":
        wl_d = nc.dram_tensor("wencL", [ACT_DIM, DICT], bf16,
                              kind="ExternalInput")
    wdecT_d = nc.dram_tensor("wdecT", [DICT // 1024, 128, 8 * ACT_DIM], bf16,
                             kind="ExternalInput")
    bdec_d = nc.dram_tensor("bdec", [1, ACT_DIM], f32, kind="ExternalInput")
    if with_benc:
        benc_d = nc.dram_tensor("benc", [1, DICT], f32, kind="ExternalInput")
    xhat_d = nc.dram_tensor("xhat", [ROWS, ACT_DIM], f32, kind="ExternalOutput")
    flags_d = nc.dram_tensor("flags", [128, NT], f32, kind="ExternalOutput")
    acts_spill = nc.dram_tensor("acts_spill", [NT, 128, DICT], f32)

    with tile.TileContext(nc) as tc:
        with tc.tile_pool(name="const", bufs=1) as constp, \
             tc.tile_pool(name="cand", bufs=NT) as candp, \
             tc.tile_pool(name="small", bufs=NT) as smallp:

            bdec_row = constp.tile([1, ACT_DIM], f32)
            nc.sync.dma_start(bdec_row[:], bdec_d.ap())
            bdec_bc = constp.tile([128, ACT_DIM], f32)
            nc.gpsimd.partition_broadcast(bdec_bc[:], bdec_row[:])

            flags_sb = constp.tile([128, NT], f32)
            # candidate array + layer-2 array at the end:
            # [chunk cands 0..CANDW | 32 group top-8s at CANDW..CANDW+L2W]
            cands = [candp.tile([128, CANDW + L2W], f32, tag="cand",
                                name=f"cand{t}") for t in range(NT)]
            taus = [smallp.tile([128, 1], f32, tag="tau", name=f"tau{t}")
                    for t in range(NT)]

            # ---------------- Phase A: encode + spill + stage-1 ----------
            with tc.tile_pool(name="xt", bufs=1) as xtp, \
                 tc.tile_pool(name="wenc", bufs=4) as wencp, \
                 tc.tile_pool(name="bounce", bufs=14) as bouncep, \
                 tc.tile_pool(name="encpsum", bufs=6, space="PSUM") as encpsp, \
                 tc.tile_pool(name="bencbc", bufs=2) as bencbcp:

                xh_sb = xtp.tile([128, NA, ROWS], wdt)
                xl_sb = xtp.tile([128, NA, ROWS], wdt)
                nc.sync.dma_start(
                    xh_sb[:], xh_d.ap().rearrange("(a p) r -> p a r", p=128))
                nc.sync.dma_start(
                    xl_sb[:], xl_d.ap().rearrange("(a p) r -> p a r", p=128))

                for sc in range(NSC):
                    whch = wencp.tile([128, NA, 512], wdt, tag="wh",
                                      name=f"wh{sc}")
                    nc.sync.dma_start(
                        whch[:],
                        wh_d.ap()[:, sc * 512:(sc + 1) * 512]
                        .rearrange("(a p) c -> p a c", p=128))
                    if mode == "bf16x3":
                        wlch = wencp.tile([128, NA, 512], bf16, tag="wl",
                                          name=f"wl{sc}")
                        nc.sync.dma_start(
                            wlch[:],
                            wl_d.ap()[:, sc * 512:(sc + 1) * 512]
                            .rearrange("(a p) c -> p a c", p=128))
                    if with_benc:
                        br = bencbcp.tile([1, 512], f32, tag="br")
                        nc.sync.dma_start(
                            br[:], benc_d.ap()[0:1, sc * 512:(sc + 1) * 512])
                        bb = bencbcp.tile([128, 512], f32, tag="bb")
                        nc.gpsimd.partition_broadcast(bb[:], br[:])
                    for t in range(NT):
                        ps = encpsp.tile([128, 512], f32, tag="eps")
                        rt = slice(t * 128, (t + 1) * 128)
                        if mode == "bf16x3":
                            n_mm = 3 * NA
                            i = 0
                            for a in range(NA):
                                for w in (whch, wlch):
                                    nc.tensor.matmul(
                                        ps[:], xh_sb[:, a, rt], w[:, a, :],
                                        start=(i == 0), stop=(i == n_mm - 1))
                                    i += 1
                            for a in range(NA):
                                nc.tensor.matmul(
                                    ps[:], xl_sb[:, a, rt], whch[:, a, :],
                                    start=(i == 0), stop=(i == n_mm - 1))
                                i += 1
                        else:
                            n_mm = 2 * NA
                            i = 0
                            for a in range(NA):
                                for xs in (xh_sb, xl_sb):
                                    nc.tensor.matmul(
                                        ps[:], xs[:, a, rt], whch[:, a, :],
                                        start=(i == 0), stop=(i == n_mm - 1))
                                    i += 1
                        bo = bouncep.tile([128, 512], f32, tag="bo")
                        if with_benc:
                            nc.vector.tensor_tensor(bo[:], ps[:], bb[:],
                                                    op=mybir.AluOpType.add)
                            nc.scalar.activation(
                                bo[:], bo[:], mybir.ActivationFunctionType.Relu)
                        else:
                            nc.scalar.activation(
                                bo[:], ps[:], mybir.ActivationFunctionType.Relu)
                        nc.sync.dma_start(
                            acts_spill.ap()[t, :, sc * 512:(sc + 1) * 512],
                            bo[:])
                        for cc in range(512 // CH):
                            c = sc * (512 // CH) + cc
                            nc.vector.max(
                                cands[t][:, c * 8:(c + 1) * 8],
                                bo[:, cc * CH:(cc + 1) * CH])
                    # layer-2: reduce any 3-chunk group completed at this sc
                    for g in range(NGRP):
                        if (3 * g + 2) // 2 == sc:
                            for t in range(NT):
                                nc.vector.max(
                                    cands[t][:, CANDW + g * 8:
                                             CANDW + (g + 1) * 8],
                                    cands[t][:, g * 24:(g + 1) * 24])

            # -------- Phases B+C+D: threshold, mask/transpose, decode ----
            # Granule: 1024 F-columns (half a 2048 block, = one g-group of
            # 8 decoder f-chunks).  One-block lookahead on the reload ->
            # mask -> transpose chain keeps the PE fed.
            with tc.tile_pool(name="actsc", bufs=12) as actscp, \
                 tc.tile_pool(name="encb", bufs=8) as encbp, \
                 tc.tile_pool(name="enct", bufs=18) as enctp, \
                 tc.tile_pool(name="wdec", bufs=4) as wdecp, \
                 tc.tile_pool(name="decpsum", bufs=NT, space="PSUM") as decpsp, \
                 tc.tile_pool(name="outsb", bufs=2) as outp:

                ets = {}
                wds = {}

                def mask_transpose(t, blk, g):
                    ac = actscp.tile([128, 1024], f32, tag="ac",
                                     name=f"ac{t}_{blk}_{g}")
                    lo = blk * 2048 + g * 1024
                    nc.sync.dma_start(
                        ac[:], acts_spill.ap()[t, :, lo:lo + 1024])
                    eb = encbp.tile([128, 1024], bf16, tag="eb",
                                    name=f"eb{t}_{blk}_{g}")
                    nc.vector.scalar_tensor_tensor(
                        eb[:], ac[:], taus[t][:, 0:1], ac[:],
                        op0=mybir.AluOpType.is_ge,
                        op1=mybir.AluOpType.mult)
                    et = enctp.tile([128, 8, 128], bf16, tag="enct",
                                    name=f"et{t}_{blk}_{g}")
                    # 4 sub-transposes on separate DMA queues: one queue
                    # moves a [128,1024] bf16 transpose in 256B packets
                    # (~20us serial), which cannot hide under one block
                    for q in range(4):
                        nc.sync.dma_start_transpose(
                            et[:, 2 * q:2 * q + 2, :],
                            eb[:, q * 256:(q + 1) * 256])
                    ets[(t, blk, g)] = et

                def load_wd(blk):
                    for g in range(2):
                        wd = wdecp.tile([128, 8, ACT_DIM], bf16, tag="wd",
                                        name=f"wd{blk}_{g}")
                        nc.sync.dma_start(
                            wd[:].rearrange("p c a -> p (c a)"),
                            wdecT_d.ap()[blk * 2 + g, :, :])
                        wds[(blk, g)] = wd

                load_wd(0)

                for t in range(NT):
                    # stage 2: top-k of the layer-2 array; flag = any
                    # group-8th-largest above tau (group overflow)
                    c8 = smallp.tile([128, 1], f32, tag="c8", name=f"c8_{t}")
                    l2 = cands[t][:, CANDW:CANDW + L2W].rearrange(
                        "p (c e) -> p c e", e=8)
                    nc.vector.tensor_reduce(c8[:], l2[:, :, 7:8],
                                            axis=mybir.AxisListType.XY,
                                            op=mybir.AluOpType.max)
                    topv = smallp.tile([128, R8], f32, tag="topv",
                                       name=f"topv{t}")
                    for r in range(ROUNDS):
                        nc.vector.max(topv[:, r * 8:(r + 1) * 8],
                                      cands[t][:, CANDW:CANDW + L2W])
                        if r < ROUNDS - 1:
                            nc.vector.match_replace(
                                cands[t][:, CANDW:CANDW + L2W],
                                topv[:, r * 8:(r + 1) * 8],
                                cands[t][:, CANDW:CANDW + L2W], NEG)
                    nc.vector.tensor_copy(taus[t][:], topv[:, k - 1:k])
                    nc.vector.tensor_tensor(flags_sb[:, t:t + 1], c8[:],
                                            taus[t][:],
                                            op=mybir.AluOpType.is_gt)
                    for g in range(2):
                        mask_transpose(t, 0, g)

                pss = [decpsp.tile([128, ACT_DIM], f32, tag="dps",
                                   name=f"dps{t}") for t in range(NT)]
                for blk in range(NBLK):
                    if blk + 1 < NBLK:
                        load_wd(blk + 1)
                        for t in range(NT):
                            for g in range(2):
                                mask_transpose(t, blk + 1, g)
                    for g in range(2):
                        wd = wds.pop((blk, g))
                        for t in range(NT):
                            for j in range(8):
                                f = blk * 16 + g * 8 + j
                                lhsT = ets[(t, blk, g)][:, j, :]
                                st = (f == 0)
                                sp = (f == NF - 1)
                                nc.tensor.matmul(
                                    pss[t][:, 0:512], lhsT, wd[:, j, 0:512],
                                    start=st, stop=sp)
                                nc.tensor.matmul(
                                    pss[t][:, 512:ACT_DIM], lhsT,
                                    wd[:, j, 512:ACT_DIM],
                                    start=st, stop=sp)
                        for t in range(NT):
                            ets.pop((t, blk, g))
                for t in range(NT):
                    ot = outp.tile([128, ACT_DIM], f32, tag="ot",
                                   name=f"ot{t}")
                    nc.vector.tensor_tensor(ot[:], pss[t][:], bdec_bc[:],
                                            op=mybir.AluOpType.add)
                    nc.sync.dma_start(
                        xhat_d.ap()[t * 128:(t + 1) * 128, :], ot[:])
                nc.sync.dma_start(flags_d.ap(), flags_sb[:])

    nc.compile()
    return nc


def _get_program(k: int, with_benc: bool, mode: str):
    key = (k, with_benc, mode)
    if key not in _cache:
        _cache[key] = _build(k, with_benc, mode)
    return _cache[key]


def _host_repair(out, rows, x, W_enc, b_enc, W_dec, b_dec, k):
    for r in rows:
        pre = (x[r] - b_dec) @ W_enc.T + b_enc
        acts = np.maximum(pre, 0.0)
        idx = np.argsort(-acts, kind="stable")[:k]
        enc = np.zeros_like(acts)
        enc[idx] = acts[idx]
        out[r] = enc @ W_dec.T + b_dec


def run(inputs, trace=False, mode=MODE):
    from concourse.bass_utils import run_bass_kernel_spmd

    x = np.asarray(inputs["x"], dtype=np.float32)
    W_enc = np.asarray(inputs["W_enc"], dtype=np.float32)
    b_enc = np.asarray(inputs["b_enc"], dtype=np.float32)
    W_dec = np.asarray(inputs["W_dec"], dtype=np.float32)
    b_dec = np.asarray(inputs["b_dec"], dtype=np.float32)
    k = int(np.asarray(inputs["k"]))
    assert x.shape == (BATCH, ACT_DIM) and W_enc.shape == (DICT, ACT_DIM)
    assert 1 <= k <= 64

    with_benc = bool(np.any(b_enc))
    nc = _get_program(k, with_benc, mode)

    xT = np.ascontiguousarray((x - b_dec).T, dtype=np.float32)
    wencT = np.ascontiguousarray(W_enc.T, dtype=np.float32)
    if mode == "bf16x3":
        xTh = xT.astype(BF16)
        xTl = (xT - xTh.astype(np.float32)).astype(BF16)
        wencH = wencT.astype(BF16)
        wencL = (wencT - wencH.astype(np.float32)).astype(BF16)
    else:
        xTh = xT.astype(np.float16)
        xTl = (xT - xTh.astype(np.float32)).astype(np.float16)
        wencH = wencT.astype(np.float16)
        wencL = None
    wdecT = np.ascontiguousarray(W_dec.T).astype(BF16)
    # [NFG, 128, 8*ACT_DIM]: partition p of group fg holds rows of the 8
    # 128-row f-chunks, giving 12KB contiguous per-partition DMA reads
    wdec_r = np.ascontiguousarray(
        wdecT.reshape(DICT // 1024, 8, 128, ACT_DIM).transpose(0, 2, 1, 3)
        .reshape(DICT // 1024, 128, 8 * ACT_DIM))
    bdec_row = np.ascontiguousarray(b_dec.reshape(1, ACT_DIM))

    in_maps = []
    for c in range(NCORES):
        sl = slice(c * ROWS, (c + 1) * ROWS)
        m = {
            "xh": np.ascontiguousarray(xTh[:, sl]),
            "xl": np.ascontiguousarray(xTl[:, sl]),
            "wencH": wencH,
            "wdecT": wdec_r,
            "bdec": bdec_row,
        }
        if mode == "bf16x3":
            m["wencL"] = wencL
        if with_benc:
            m["benc"] = np.ascontiguousarray(b_enc.reshape(1, DICT))
        in_maps.append(m)

    res = run_bass_kernel_spmd(nc, in_maps, core_ids=list(range(NCORES)),
                               trace=trace)

    out = np.empty((BATCH, ACT_DIM), dtype=np.float32)
    flagged = []
    for c in range(NCORES):
        out[c * ROWS:(c + 1) * ROWS] = res.results[c]["xhat"]
        fl = res.results[c]["flags"]          # [128, NT]
        for t in range(NT):
            for p in np.nonzero(fl[:, t] > 0)[0]:
                flagged.append(c * ROWS + t * 128 + int(p))
    if flagged:
        _host_repair(out, flagged, x, W_enc, b_enc, W_dec, b_dec, k)
    return out, res, flagged


def kernel(**inputs) -> np.ndarray:
    out, _, _ = run(inputs)
    return out


# revision 18
# speedup vs baseline: 1.2067x; 1.2067x over previous
"""TopK sparse autoencoder forward pass on 8 TRN2 NeuronCores.

Data-parallel over the batch: each core owns 512 rows and runs an identical
program (SPMD).  Per core:

  A. encode:  acts = relu((x - b_dec) @ W_enc.T + b_enc)
     - MODE bf16x3: 3-term bf16 hi/lo split (xh@Wh + xh@Wl + xl@Wh), ~fp32
       precision -> exact top-k selection.
     - MODE fp16x2: 2-term fp16 x-split ((xh + xl) @ W16); residual x@(W -
       W16) gives acts error ~1e-4 -> a few tens of swapped rows batch-wide,
       rel err ~1.5e-2 (gate 2e-2); W streamed once at half the HBM traffic
       and 2/3 the PE time of bf16x3.
     - W streamed from HBM once; fp32 acts spilled to DRAM
     - per-256-chunk top-8 candidates extracted from drain bounces (DVE max8)
  B. topk: exact threshold tau from the candidate array via iterated
     max8 + match_replace, split hierarchically: candidates of chunks 0..79
     are reduced to a top-8R block on the DVE *while the PE still encodes*
     chunks 80..95; the post-encode stage only reduces 128+8R columns.
  C. mask: enc = (acts >= tau) * acts, cast bf16, DMA-transposed to [F, B]
  D. decode: x_hat = enc @ W_dec.T + b_dec  [bf16, encoded-stationary],
     block-pipelined over 2048-wide F blocks with one-block lookahead on
     the acts reload + mask + transpose chain (1024-wide granules) and
     W_dec prefetch, so the PE never idles between blocks (idle gaps ->
     HAM re-throttle to half clock, which is what made the un-pipelined
     version slow).

The per-chunk top-8 candidate set provably contains the true top-k unless
some 256-wide chunk holds >8 of the top-k values; that condition is detected
on-device (flag = chunk-8th-largest > tau) and the handful of flagged rows
(expected: zero) are recomputed exactly on the host.
"""

import numpy as np
import ml_dtypes

ACT_DIM = 768
DICT = 24576
BATCH = 4096
NCORES = 8
ROWS = BATCH // NCORES          # 512 rows per core
NT = ROWS // 128                # 4 row-tiles per core
CH = 256                        # stage-1 chunk width
NCH = DICT // CH                # 96 chunks
CANDW = NCH * 8                 # 768 candidates per row
NEG = -1.0e30
BF16 = ml_dtypes.bfloat16
NA = ACT_DIM // 128             # 6 K-chunks
NSC = DICT // 512               # 48 encode column-chunks
# layer-2 tau tournament: 96 chunks -> 32 groups of 3 chunks (768 dict
# cols); per-group top-8 extracted incrementally during encode, stage 2
# reduces the 256-wide layer-2 array after encode.  A group holding >8
# of the top-k is detected via the flag (expected ~30 rows batch-wide,
# host-repaired exactly).
GRP = 3                         # chunks per layer-2 group
NGRP = NCH // GRP               # 32 groups
L2W = NGRP * 8                  # 256 layer-2 candidate cols
NBLK = DICT // 2048             # 12 C/D blocks
NF = DICT // 128                # 192 decoder f-chunks

MODE = "fp16x2"                 # "bf16x3" | "fp16x2"

_cache = {}


def _build(k: int, with_benc: bool, mode: str):
    import concourse.bass as bass
    import concourse.mybir as mybir
    from concourse import bacc
    from concourse import tile

    f32 = mybir.dt.float32
    bf16 = mybir.dt.bfloat16
    fp16 = mybir.dt.float16
    ROUNDS = (k + 7) // 8
    R8 = 8 * ROUNDS

    nc = bacc.Bacc("TRN2", target_bir_lowering=False, debug=False,
                   num_devices=NCORES)

    wdt = bf16 if mode == "bf16x3" else fp16
    xh_d = nc.dram_tensor("xh", [ACT_DIM, ROWS], wdt, kind="ExternalInput")
    xl_d = nc.dram_tensor("xl", [ACT_DIM, ROWS], wdt, kind="ExternalInput")
    wh_d = nc.dram_tensor("wencH", [ACT_DIM, DICT], wdt, kind="ExternalInput")
    if mode == "bf16x3":
        wl_d = nc.dram_tensor("wencL", [ACT_DIM, DICT], bf16,
                              kind="ExternalInput")
    wdecT_d = nc.dram_tensor("wdecT", [DICT // 1024, 128, 8 * ACT_DIM], bf16,
                             kind="ExternalInput")
    bdec_d = nc.dram_tensor("bdec", [1, ACT_DIM], f32, kind="ExternalInput")
    if with_benc:
        benc_d = nc.dram_tensor("benc", [1, DICT], f32, kind="ExternalInput")
    xhat_d = nc.dram_tensor("xhat", [ROWS, ACT_DIM], f32, kind="ExternalOutput")
    flags_d = nc.dram_tensor("flags", [128, NT], f32, kind="ExternalOutput")
    acts_spill = nc.dram_tensor("acts_spill", [NT, 128, DICT], f32)

    with tile.TileContext(nc) as tc:
        with tc.tile_pool(name="const", bufs=1) as constp, \
             tc.tile_pool(name="cand", bufs=NT) as candp, \
             tc.tile_pool(name="small", bufs=NT) as smallp:

            bdec_row = constp.tile([1, ACT_DIM], f32)
            nc.sync.dma_start(bdec_row[:], bdec_d.ap())
            bdec_bc = constp.tile([128, ACT_DIM], f32)
            nc.gpsimd.partition_broadcast(bdec_bc[:], bdec_row[:])

            flags_sb = constp.tile([128, NT], f32)
            # candidate array + layer-2 array at the end:
            # [chunk cands 0..CANDW | 32 group top-8s at CANDW..CANDW+L2W]
            cands = [candp.tile([128, CANDW + L2W], f32, tag="cand",
                                name=f"cand{t}") for t in range(NT)]
            taus = [smallp.tile([128, 1], f32, tag="tau", name=f"tau{t}")
                    for t in range(NT)]

            # ---------------- Phase A: encode + spill + stage-1 ----------
            with tc.tile_pool(name="xt", bufs=1) as xtp, \
                 tc.tile_pool(name="wenc", bufs=4) as wencp, \
                 tc.tile_pool(name="bounce", bufs=14) as bouncep, \
                 tc.tile_pool(name="encpsum", bufs=6, space="PSUM") as encpsp, \
                 tc.tile_pool(name="bencbc", bufs=2) as bencbcp:

                xh_sb = xtp.tile([128, NA, ROWS], wdt)
                xl_sb = xtp.tile([128, NA, ROWS], wdt)
                nc.sync.dma_start(
                    xh_sb[:], xh_d.ap().rearrange("(a p) r -> p a r", p=128))
                nc.sync.dma_start(
                    xl_sb[:], xl_d.ap().rearrange("(a p) r -> p a r", p=128))

                for sc in range(NSC):
                    whch = wencp.tile([128, NA, 512], wdt, tag="wh",
                                      name=f"wh{sc}")
                    nc.sync.dma_start(
                        whch[:],
                        wh_d.ap()[:, sc * 512:(sc + 1) * 512]
                        .rearrange("(a p) c -> p a c", p=128))
                    if mode == "bf16x3":
                        wlch = wencp.tile([128, NA, 512], bf16, tag="wl",
                                          name=f"wl{sc}")
                        nc.sync.dma_start(
                            wlch[:],
                            wl_d.ap()[:, sc * 512:(sc + 1) * 512]
                            .rearrange("(a p) c -> p a c", p=128))
                    if with_benc:
                        br = bencbcp.tile([1, 512], f32, tag="br")
                        nc.sync.dma_start(
                            br[:], benc_d.ap()[0:1, sc * 512:(sc + 1) * 512])
                        bb = bencbcp.tile([128, 512], f32, tag="bb")
                        nc.gpsimd.partition_broadcast(bb[:], br[:])
                    for t in range(NT):
                        ps = encpsp.tile([128, 512], f32, tag="eps")
                        rt = slice(t * 128, (t + 1) * 128)
                        if mode == "bf16x3":
                            n_mm = 3 * NA
                            i = 0
                            for a in range(NA):
                                for w in (whch, wlch):
                                    nc.tensor.matmul(
                                        ps[:], xh_sb[:, a, rt], w[:, a, :],
                                        start=(i == 0), stop=(i == n_mm - 1))
                                    i += 1
                            for a in range(NA):
                                nc.tensor.matmul(
                                    ps[:], xl_sb[:, a, rt], whch[:, a, :],
                                    start=(i == 0), stop=(i == n_mm - 1))
                                i += 1
                        else:
                            n_mm = 2 * NA
                            i = 0
                            for a in range(NA):
                                for xs in (xh_sb, xl_sb):
                                    nc.tensor.matmul(
                                        ps[:], xs[:, a, rt], whch[:, a, :],
                                        start=(i == 0), stop=(i == n_mm - 1))
                                    i += 1
                        bo = bouncep.tile([128, 512], f32, tag="bo")
                        if with_benc:
                            nc.vector.tensor_tensor(bo[:], ps[:], bb[:],
                                                    op=mybir.AluOpType.add)
                            nc.scalar.activation(
                                bo[:], bo[:], mybir.ActivationFunctionType.Relu)
                        else:
                            nc.scalar.activation(
                                bo[:], ps[:], mybir.ActivationFunctionType.Relu)
                        nc.sync.dma_start(
                            acts_spill.ap()[t, :, sc * 512:(sc + 1) * 512],
                            bo[:])
                        for cc in range(512 // CH):
                            c = sc * (512 // CH) + cc
                            nc.vector.max(
                                cands[t][:, c * 8:(c + 1) * 8],
                                bo[:, cc * CH:(cc + 1) * CH])
                    # layer-2: reduce any 3-chunk group completed at this sc
                    for g in range(NGRP):
                        if (3 * g + 2) // 2 == sc:
                            for t in range(NT):
                                nc.vector.max(
                                    cands[t][:, CANDW + g * 8:
                                             CANDW + (g + 1) * 8],
                                    cands[t][:, g * 24:(g + 1) * 24])

            # -------- Phases B+C+D: threshold, mask/transpose, decode ----
            # Granule: 1024 F-columns (half a 2048 block, = one g-group of
            # 8 decoder f-chunks).  One-block lookahead on the reload ->
            # mask -> transpose chain keeps the PE fed.
            with tc.tile_pool(name="actsc", bufs=10) as actscp, \
                 tc.tile_pool(name="encb", bufs=16) as encbp, \
                 tc.tile_pool(name="enct", bufs=26) as enctp, \
                 tc.tile_pool(name="wdec", bufs=4) as wdecp, \
                 tc.tile_pool(name="decpsum", bufs=NT, space="PSUM") as decpsp, \
                 tc.tile_pool(name="outsb", bufs=2) as outp:

                ets = {}
                wds = {}

                def mask_transpose(t, blk, g):
                    ac = actscp.tile([128, 1024], f32, tag="ac",
                                     name=f"ac{t}_{blk}_{g}")
                    lo = blk * 2048 + g * 1024
                    nc.sync.dma_start(
                        ac[:], acts_spill.ap()[t, :, lo:lo + 1024])
                    eb = encbp.tile([128, 1024], bf16, tag="eb",
                                    name=f"eb{t}_{blk}_{g}")
                    nc.vector.scalar_tensor_tensor(
                        eb[:], ac[:], taus[t][:, 0:1], ac[:],
                        op0=mybir.AluOpType.is_ge,
                        op1=mybir.AluOpType.mult)
                    et = enctp.tile([128, 8, 128], bf16, tag="enct",
                                    name=f"et{t}_{blk}_{g}")
                    # transposes move in 256B packets (~20us of DMA-engine
                    # time per granule); issue them on the Activation HWDGE
                    # queue so the bulk ac/wd loads on the sync queue never
                    # wait behind them, and give them 2 blocks of lookahead
                    nc.scalar.dma_start_transpose(et[:], eb[:])
                    ets[(t, blk, g)] = et

                def load_wd(blk):
                    for g in range(2):
                        wd = wdecp.tile([128, 8, ACT_DIM], bf16, tag="wd",
                                        name=f"wd{blk}_{g}")
                        nc.sync.dma_start(
                            wd[:].rearrange("p c a -> p (c a)"),
                            wdecT_d.ap()[blk * 2 + g, :, :])
                        wds[(blk, g)] = wd

                load_wd(0)

                for t in range(NT):
                    # stage 2: top-k of the layer-2 array; flag = any
                    # group-8th-largest above tau (group overflow)
                    c8 = smallp.tile([128, 1], f32, tag="c8", name=f"c8_{t}")
                    l2 = cands[t][:, CANDW:CANDW + L2W].rearrange(
                        "p (c e) -> p c e", e=8)
                    nc.vector.tensor_reduce(c8[:], l2[:, :, 7:8],
                                            axis=mybir.AxisListType.XY,
                                            op=mybir.AluOpType.max)
                    topv = smallp.tile([128, R8], f32, tag="topv",
                                       name=f"topv{t}")
                    for r in range(ROUNDS):
                        nc.vector.max(topv[:, r * 8:(r + 1) * 8],
                                      cands[t][:, CANDW:CANDW + L2W])
                        if r < ROUNDS - 1:
                            nc.vector.match_replace(
                                cands[t][:, CANDW:CANDW + L2W],
                                topv[:, r * 8:(r + 1) * 8],
                                cands[t][:, CANDW:CANDW + L2W], NEG)
                    nc.vector.tensor_copy(taus[t][:], topv[:, k - 1:k])
                    nc.vector.tensor_tensor(flags_sb[:, t:t + 1], c8[:],
                                            taus[t][:],
                                            op=mybir.AluOpType.is_gt)
                    for blk in range(2):
                        for g in range(2):
                            mask_transpose(t, blk, g)

                pss = [decpsp.tile([128, ACT_DIM], f32, tag="dps",
                                   name=f"dps{t}") for t in range(NT)]
                for blk in range(NBLK):
                    if blk + 1 < NBLK:
                        load_wd(blk + 1)
                    if blk + 2 < NBLK:
                        for t in range(NT):
                            for g in range(2):
                                mask_transpose(t, blk + 2, g)
                    for g in range(2):
                        wd = wds.pop((blk, g))
                        for t in range(NT):
                            for j in range(8):
                                f = blk * 16 + g * 8 + j
                                lhsT = ets[(t, blk, g)][:, j, :]
                                st = (f == 0)
                                sp = (f == NF - 1)
                                nc.tensor.matmul(
                                    pss[t][:, 0:512], lhsT, wd[:, j, 0:512],
                                    start=st, stop=sp)
                                nc.tensor.matmul(
                                    pss[t][:, 512:ACT_DIM], lhsT,
                                    wd[:, j, 512:ACT_DIM],
                                    start=st, stop=sp)
                        for t in range(NT):
                            ets.pop((t, blk, g))
                for t in range(NT):
                    ot = outp.tile([128, ACT_DIM], f32, tag="ot",
                                   name=f"ot{t}")
                    nc.vector.tensor_tensor(ot[:], pss[t][:], bdec_bc[:],
                                            op=mybir.AluOpType.add)
                    nc.sync.dma_start(
                        xhat_d.ap()[t * 128:(t + 1) * 128, :], ot[:])
                nc.sync.dma_start(flags_d.ap(), flags_sb[:])

    nc.compile()
    return nc


def _get_program(k: int, with_benc: bool, mode: str):
    key = (k, with_benc, mode)
    if key not in _cache:
        _cache[key] = _build(k, with_benc, mode)
    return _cache[key]


def _host_repair(out, rows, x, W_enc, b_enc, W_dec, b_dec, k):
    for r in rows:
        pre = (x[r] - b_dec) @ W_enc.T + b_enc
        acts = np.maximum(pre, 0.0)
        idx = np.argsort(-acts, kind="stable")[:k]
        enc = np.zeros_like(acts)
        enc[idx] = acts[idx]
        out[r] = enc @ W_dec.T + b_dec


def run(inputs, trace=False, mode=MODE):
    from concourse.bass_utils import run_bass_kernel_spmd

    x = np.asarray(inputs["x"], dtype=np.float32)
    W_enc = np.asarray(inputs["W_enc"], dtype=np.float32)
    b_enc = np.asarray(inputs["b_enc"], dtype=np.float32)
    W_dec = np.asarray(inputs["W_dec"], dtype=np.float32)
    b_dec = np.asarray(inputs["b_dec"], dtype=np.float32)
    k = int(np.asarray(inputs["k"]))
    assert x.shape == (BATCH, ACT_DIM) and W_enc.shape == (DICT, ACT_DIM)
    assert 1 <= k <= 64

    with_benc = bool(np.any(b_enc))
    nc = _get_program(k, with_benc, mode)

    xT = np.ascontiguousarray((x - b_dec).T, dtype=np.float32)
    wencT = np.ascontiguousarray(W_enc.T, dtype=np.float32)
    if mode == "bf16x3":
        xTh = xT.astype(BF16)
        xTl = (xT - xTh.astype(np.float32)).astype(BF16)
        wencH = wencT.astype(BF16)
        wencL = (wencT - wencH.astype(np.float32)).astype(BF16)
    else:
        xTh = xT.astype(np.float16)
        xTl = (xT - xTh.astype(np.float32)).astype(np.float16)
        wencH = wencT.astype(np.float16)
        wencL = None
    wdecT = np.ascontiguousarray(W_dec.T).astype(BF16)
    # [NFG, 128, 8*ACT_DIM]: partition p of group fg holds rows of the 8
    # 128-row f-chunks, giving 12KB contiguous per-partition DMA reads
    wdec_r = np.ascontiguousarray(
        wdecT.reshape(DICT // 1024, 8, 128, ACT_DIM).transpose(0, 2, 1, 3)
        .reshape(DICT // 1024, 128, 8 * ACT_DIM))
    bdec_row = np.ascontiguousarray(b_dec.reshape(1, ACT_DIM))

    in_maps = []
    for c in range(NCORES):
        sl = slice(c * ROWS, (c + 1) * ROWS)
        m = {
            "xh": np.ascontiguousarray(xTh[:, sl]),
            "xl": np.ascontiguousarray(xTl[:, sl]),
            "wencH": wencH,
            "wdecT": wdec_r,
            "bdec": bdec_row,
        }
        if mode == "bf16x3":
            m["wencL"] = wencL
        if with_benc:
            m["benc"] = np.ascontiguousarray(b_enc.reshape(1, DICT))
        in_maps.append(m)

    res = run_bass_kernel_spmd(nc, in_maps, core_ids=list(range(NCORES)),
                               trace=trace)

    out = np.empty((BATCH, ACT_DIM), dtype=np.float32)
    flagged = []
    for c in range(NCORES):
        out[c * ROWS:(c + 1) * ROWS] = res.results[c]["xhat"]
        fl = res.results[c]["flags"]          # [128, NT]
        for t in range(NT):
            for p in np.nonzero(fl[:, t] > 0)[0]:
                flagged.append(c * ROWS + t * 128 + int(p))
    if flagged:
        _host_repair(out, flagged, x, W_enc, b_enc, W_dec, b_dec, k)
    return out, res, flagged


def kernel(**inputs) -> np.ndarray:
    out, _, _ = run(inputs)
    return out
